# revision 10
# baseline (speedup 1.0000x reference)
"""Trainium2 Bass kernel for nn_ContrastiveDist (supervised contrastive loss).

Math
----
The (n,n) distance/weight matrices collapse to per-class statistics.  With
classes c = 0..15, per-class count cnt[c], feature sums C[c,:], squared-norm
sums SqS[c], global sums Ftot / SSall:

    alpha[c] = 1/(cnt[c]-1+eps),  beta[c] = 1/(n-cnt[c]+eps)
    loss_i   = f_i . R[c_i] + sq_i*P[c_i] + (Q[c_i]+M)
      R[c,:] = 2*beta*(Ftot-C[c]) - 2*alpha*C[c]
      P[c]   = alpha*cnt - beta*(n-cnt)
      Q[c]   = alpha*SqS[c] - beta*(SSall-SqS[c])
    result   = sum(relu(loss_i)*valid_i) / max(sum(valid_i), 1)

valid_i = (cnt[c_i] >= 2) is folded into the coefficients (R/P/QM rows of
invalid classes zeroed -> relu(loss)=0 there).

Device pipeline (single-chain bf16, ~5e-5 rel err vs f32 reference):
  1. stats:  statsT(128d,16c) = sum_t fh_tile^T @ onehot_tile  (64-matmul
     PSUM accumulation chain, lands directly in the transposed layout needed
     as dot-phase weights), overlapped with the feature DMA.
  2. cnt-only coefficients (alpha/beta/vmask/P and their 128-partition
     broadcast via a ones(1,128) rank-1 matmul) are computed EARLY from the
     one-hot column sums; only the SqS-dependent QM and the stats-dependent
     RT remain on the post-DMA critical path.
  3. loss:   per 512-col chunk, PSUM = RT^T @ fT + P128^T @ fT^2  (the second
     matmul realizes P[c]*sq_i since sum_d fT^2[d,i] = sq_i), then
     relu(PSUM + QM[c]) on the scalar engine and mask*accumulate on vector.
Total HBM traffic ~4.7MB/core (bf16 features in rows + transposed layouts,
prebuilt one-hots); every core computes redundantly (no collectives).
"""

import numpy as np
import ml_dtypes

import concourse.bacc as bacc
import concourse.tile as tile
import concourse.mybir as mybir
from concourse.bass_utils import run_bass_kernel_spmd

N, D, K, NCORES = 8192, 128, 16, 8
T = N // 128               # 64 row-tiles of 128
NCH = 16                   # dot chunks of 512 cols
CH = N // NCH
FCH = 4                    # DMA / square chunking (2048 cols each)
EPS, MARGIN = 1e-6, 10.0
F32 = mybir.dt.float32
BF16 = mybir.dt.bfloat16
Alu = mybir.AluOpType
Act = mybir.ActivationFunctionType
AxX = mybir.AxisListType.X

_CACHE: dict = {}


def _build():
    if "nc" in _CACHE:
        return _CACHE["nc"]

    nc = bacc.Bacc("TRN2", target_bir_lowering=False, debug=False, num_devices=NCORES)
    fhr = nc.dram_tensor("fhr", [128, T * D], BF16, kind="ExternalInput").ap()
    ftr = nc.dram_tensor("ftr", [128, N], BF16, kind="ExternalInput").ap()
    eohr = nc.dram_tensor("eohr", [128, T * K], BF16, kind="ExternalInput").ap()
    eoht = nc.dram_tensor("eoht", [K, N], BF16, kind="ExternalInput").ap()
    res = nc.dram_tensor("res", [1, 1], F32, kind="ExternalOutput").ap()

    with tile.TileContext(nc) as tc:
        with (
            tc.tile_pool(name="sb", bufs=1) as sb,
            tc.tile_pool(name="ps", bufs=1, space="PSUM") as ps,
        ):
            # ---------------- loads (3 dispatch rings, fh first) ----------------
            eohs = sb.tile([128, T * K], BF16)
            fh = sb.tile([128, T * D], BF16)
            ft = sb.tile([128, N], BF16)
            eohts = sb.tile([K, N], BF16)
            FC = T * D // FCH
            FT = N // FCH
            nc.gpsimd.dma_start(eohs[:], eohr)
            nc.sync.dma_start(fh[:, 0 * FC:1 * FC], fhr[:, 0 * FC:1 * FC])
            nc.scalar.dma_start(fh[:, 1 * FC:2 * FC], fhr[:, 1 * FC:2 * FC])
            nc.sync.dma_start(fh[:, 2 * FC:3 * FC], fhr[:, 2 * FC:3 * FC])
            nc.scalar.dma_start(fh[:, 3 * FC:4 * FC], fhr[:, 3 * FC:4 * FC])
            nc.gpsimd.dma_start(eohts[:], eoht)
            nc.sync.dma_start(ft[:, 0 * FT:1 * FT], ftr[:, 0 * FT:1 * FT])
            nc.scalar.dma_start(ft[:, 1 * FT:2 * FT], ftr[:, 1 * FT:2 * FT])
            nc.sync.dma_start(ft[:, 2 * FT:3 * FT], ftr[:, 2 * FT:3 * FT])
            nc.scalar.dma_start(ft[:, 3 * FT:4 * FT], ftr[:, 3 * FT:4 * FT])

            fh3 = fh.rearrange("p (t d) -> p t d", d=D)
            eoh3 = eohs.rearrange("p (t c) -> p t c", c=K)
            eoh3c = eohs.rearrange("p (t c) -> p c t", c=K)

            ones128 = sb.tile([128, 1], F32)
            nc.gpsimd.memset(ones128[:], 1.0)
            ones1 = sb.tile([1, 128], F32)
            nc.gpsimd.memset(ones1[:], 1.0)

            # ---------------- early: cnt and cnt-only coefficients ----------------
            cntpart = sb.tile([128, K], F32)
            nc.vector.tensor_reduce(cntpart[:], eoh3c, axis=AxX, op=Alu.add)
            cntP = ps.tile([1, K], F32, tag="smallP", bufs=2, name="cntP")
            nc.tensor.matmul(cntP[:], ones128[:], cntpart[:], start=True, stop=True)
            cntf = sb.tile([1, K], F32)
            nc.vector.tensor_copy(cntf[:], cntP[:])

            alpha = sb.tile([1, K], F32)
            nc.vector.tensor_scalar(alpha[:], cntf[:], EPS - 1.0, None, op0=Alu.add)
            nc.vector.reciprocal(alpha[:], alpha[:])
            beta = sb.tile([1, K], F32)
            nc.vector.tensor_scalar(beta[:], cntf[:], -1.0, float(N) + EPS,
                                    op0=Alu.mult, op1=Alu.add)
            nc.vector.reciprocal(beta[:], beta[:])
            vmask = sb.tile([1, K], F32)
            nc.vector.tensor_scalar(vmask[:], cntf[:], 1.5, None, op0=Alu.is_ge)
            nmc = sb.tile([1, K], F32)
            nc.vector.tensor_scalar(nmc[:], cntf[:], -1.0, float(N),
                                    op0=Alu.mult, op1=Alu.add)        # N-cnt
            nc.vector.tensor_tensor(nmc[:], nmc[:], beta[:], op=Alu.mult)
            pf = sb.tile([1, K], F32)
            nc.vector.tensor_tensor(pf[:], cntf[:], alpha[:], op=Alu.mult)
            nc.vector.tensor_tensor(pf[:], pf[:], nmc[:], op=Alu.subtract)

            cpack = sb.tile([1, 3 * K], F32)
            nc.vector.tensor_scalar(cpack[:, 0:K], beta[:], 2.0, None, op0=Alu.mult)
            nc.vector.tensor_scalar(cpack[:, K:2 * K], alpha[:], -2.0, None,
                                    op0=Alu.mult)
            nc.vector.tensor_tensor(cpack[:, 2 * K:3 * K], pf[:], vmask[:],
                                    op=Alu.mult)                       # P*vm
            vm2 = cpack[:, 0:2 * K].rearrange("o (a c) -> o a c", c=K)
            vmb = vmask.unsqueeze(1).broadcast_to((1, 2, K))
            nc.vector.tensor_tensor(vm2[:, :, :], vm2, vmb, op=Alu.mult)

            # ---------------- sq_i = sum_d h^2 (rows layout) ----------------
            sqdb = sb.tile([128, T], BF16)
            TPC = T // FCH
            for g in range(FCH):
                fsq = sb.tile([128, TPC * D], BF16, tag="fsq", bufs=2, name=f"fsq{g}")
                fsq3 = fsq.rearrange("p (t d) -> p t d", d=D)
                nc.scalar.activation(fsq[:], fh[:, g * FC:(g + 1) * FC], Act.Square)
                with nc.allow_low_precision(reason="bf16 sq feeds SqS only"):
                    nc.vector.tensor_reduce(sqdb[:, g * TPC:(g + 1) * TPC], fsq3,
                                            axis=AxX, op=Alu.add)

            # ---------------- ft2 = fT^2 split across Pool and Act ----------------
            ft2 = sb.tile([128, N], BF16)
            for g in (0, 1):
                nc.gpsimd.tensor_tensor(ft2[:, g * FT:(g + 1) * FT],
                                        ft[:, g * FT:(g + 1) * FT],
                                        ft[:, g * FT:(g + 1) * FT], op=Alu.mult)
            for g in (2, 3):
                nc.scalar.activation(ft2[:, g * FT:(g + 1) * FT],
                                     ft[:, g * FT:(g + 1) * FT], Act.Square)

            # ---------------- statsT chain (+ interleaved bcast matmul) ----------
            statsP = ps.tile([128, K], F32)
            bcP = ps.tile([128, 3 * K], F32)
            for t in range(T):
                nc.tensor.matmul(statsP[:], fh3[:, t, :], eoh3[:, t, :],
                                 start=(t == 0), stop=(t == T - 1),
                                 skip_group_check=True)
                if t == 31:
                    nc.tensor.matmul(bcP[:], ones1[:], cpack[:], start=True,
                                     stop=True, skip_group_check=True)

            # ---------------- SqS (vector + ones matmul) ----------------
            prods = sb.tile([128, K * T], F32)
            prods3 = prods.rearrange("p (c t) -> p c t", t=T)
            sqb3 = sqdb.unsqueeze(1).broadcast_to((128, K, T))
            nc.vector.tensor_tensor(prods3[:, :, :], eoh3c, sqb3, op=Alu.mult)
            sqspart = sb.tile([128, K], F32)
            nc.vector.tensor_reduce(sqspart[:], prods3, axis=AxX, op=Alu.add)
            csP = ps.tile([1, K], F32, tag="smallP", bufs=2, name="csP")
            nc.tensor.matmul(csP[:], ones128[:], sqspart[:], start=True, stop=True)
            SqS = sb.tile([1, K], F32)
            nc.vector.tensor_copy(SqS[:], csP[:])

            # ---------------- QM (needs SqS) ----------------
            ssall = sb.tile([1, 1], F32)
            nc.vector.tensor_reduce(ssall[:], SqS[:], axis=AxX, op=Alu.add)
            t1 = sb.tile([1, K], F32)
            nc.vector.scalar_tensor_tensor(t1[:], SqS[:], -1.0,
                                           ssall.broadcast_to((1, K)),
                                           op0=Alu.mult, op1=Alu.add)  # SSall-SqS
            nc.vector.tensor_tensor(t1[:], t1[:], beta[:], op=Alu.mult)
            qm = sb.tile([1, K], F32)
            nc.vector.tensor_tensor(qm[:], SqS[:], alpha[:], op=Alu.mult)
            nc.vector.scalar_tensor_tensor(qm[:], qm[:], MARGIN, t1[:],
                                           op0=Alu.add, op1=Alu.subtract)
            nc.vector.tensor_tensor(qm[:], qm[:], vmask[:], op=Alu.mult)
            qmtP = ps.tile([K, 1], F32, tag="smallP", bufs=2, name="qmtP")
            nc.tensor.matmul(qmtP[:], qm[:], ones1[:, 0:1], start=True, stop=True)
            qm16 = sb.tile([K, 1], F32)
            nc.vector.tensor_copy(qm16[:], qmtP[:])

            # ---------------- RT (needs stats) ----------------
            statsS = sb.tile([128, K], F32)
            nc.vector.tensor_copy(statsS[:], statsP[:])
            ftot = sb.tile([128, 1], F32)
            nc.vector.tensor_reduce(ftot[:], statsS[:], axis=AxX, op=Alu.add)
            rtf = sb.tile([128, K], F32)
            nc.vector.tensor_tensor(rtf[:], ftot.broadcast_to((128, K)), statsS[:],
                                    op=Alu.subtract)                   # Ftot-C^T
            nc.vector.tensor_tensor(rtf[:], rtf[:], bcP[:, 0:K], op=Alu.mult)
            tmp2 = sb.tile([128, K], F32)
            nc.vector.tensor_tensor(tmp2[:], statsS[:], bcP[:, K:2 * K], op=Alu.mult)
            rts = sb.tile([128, K], BF16)
            nc.vector.tensor_tensor(rts[:], rtf[:], tmp2[:], op=Alu.add)
            p128s = sb.tile([128, K], BF16)
            nc.vector.tensor_copy(p128s[:], bcP[:, 2 * K:3 * K])

            # ---------------- loss chunks ----------------
            partials = sb.tile([K, NCH], F32)
            for k in range(NCH):
                dP = ps.tile([K, CH], F32, tag="dpsum", bufs=3, name=f"dP{k}")
                nc.tensor.matmul(dP[:], rts[:], ft[:, k * CH:(k + 1) * CH],
                                 start=True, stop=False)
                nc.tensor.matmul(dP[:], p128s[:], ft2[:, k * CH:(k + 1) * CH],
                                 start=False, stop=True)
                mskd = sb.tile([K, CH], BF16, tag="mskd", bufs=3, name=f"m{k}")
                nc.scalar.activation(mskd[:], dP[:], Act.Relu, bias=qm16[:])
                scr = sb.tile([K, CH], BF16, tag="scr", bufs=3, name=f"s{k}")
                nc.vector.scalar_tensor_tensor(scr[:], mskd[:], 0.0,
                                               eohts[:, k * CH:(k + 1) * CH],
                                               op0=Alu.add, op1=Alu.mult,
                                               accum_out=partials[:, k:k + 1])

            # ---------------- final reduction ----------------
            numP = ps.tile([1, NCH], F32, tag="smallP", bufs=2, name="numP")
            nc.tensor.matmul(numP[:], ones128[0:K, :], partials[:],
                             start=True, stop=True)
            num = sb.tile([1, 1], F32)
            nc.vector.tensor_reduce(num[:], numP[:], axis=AxX, op=Alu.add)
            dv = sb.tile([1, K], F32)
            nc.vector.tensor_tensor(dv[:], cntf[:], vmask[:], op=Alu.mult)
            den = sb.tile([1, 1], F32)
            nc.vector.tensor_reduce(den[:], dv[:], axis=AxX, op=Alu.add)
            nc.vector.tensor_scalar(den[:], den[:], 1.0, None, op0=Alu.max)
            nc.vector.reciprocal(den[:], den[:])
            resS = sb.tile([1, 1], F32)
            nc.vector.tensor_tensor(resS[:], num[:], den[:], op=Alu.mult)
            nc.sync.dma_start(res, resS[:])

    nc.compile()
    _CACHE["nc"] = nc
    return nc


def _make_in_maps(features, labels):
    feats = np.ascontiguousarray(np.asarray(features, dtype=np.float32))
    lab = np.ascontiguousarray(np.asarray(labels)).astype(np.int64)
    bf = ml_dtypes.bfloat16

    oh = (lab[:, None] == np.arange(K, dtype=np.int64)[None, :]).astype(bf)  # (N, K)
    one = {
        "fhr": np.ascontiguousarray(
            feats.reshape(T, 128, D).transpose(1, 0, 2).reshape(128, T * D)
        ).astype(bf),
        "ftr": np.ascontiguousarray(feats.T).astype(bf),
        "eohr": np.ascontiguousarray(
            oh.reshape(T, 128, K).transpose(1, 0, 2).reshape(128, T * K)),
        "eoht": np.ascontiguousarray(oh.T),
    }
    return [dict(one) for _ in range(NCORES)]


def kernel(features, labels):
    nc = _build()
    in_maps = _make_in_maps(features, labels)
    out = run_bass_kernel_spmd(nc, in_maps, core_ids=list(range(NCORES)))
    return np.float32(out.results[0]["res"][0, 0])


# revision 11
# speedup vs baseline: 1.1609x; 1.1609x over previous
"""Trainium2 Bass kernel for nn_ContrastiveDist (supervised contrastive loss).

Math
----
The (n,n) distance/weight matrices collapse to per-class statistics.  With
classes c = 0..15, per-class count cnt[c], feature sums C[c,:], squared-norm
sums SqS[c], global sums Ftot / SSall:

    alpha[c] = 1/(cnt[c]-1+eps),  beta[c] = 1/(n-cnt[c]+eps)
    loss_i   = f_i . R[c_i] + sq_i*P[c_i] + (Q[c_i]+M)
      R[c,:] = 2*beta*(Ftot-C[c]) - 2*alpha*C[c]
      P[c]   = alpha*cnt - beta*(n-cnt)
      Q[c]   = alpha*SqS[c] - beta*(SSall-SqS[c])
    result   = sum(relu(loss_i)*valid_i) / max(sum(valid_i), 1)

valid_i = (cnt[c_i] >= 2) is folded into the coefficients (R/P/QM rows of
invalid classes zeroed -> relu(loss)=0 there).

Device pipeline (single-chain bf16, ~5e-5 rel err vs f32 reference):
  1. two interleaved 64-matmul PSUM chains over the row tiles produce
     statsT(128d,16c) = sum_t fh_t^T @ onehot_t  and
     sqstatsT(128d,16c) = sum_t (fh_t^2)^T @ onehot_t  (SqS[c] = its column
     sums via a ones(128,1) matmul), both overlapped with the feature DMA.
  2. cnt-only coefficients (alpha/beta/vmask/P and their 128-partition
     broadcast via a ones(1,128) rank-1 matmul) are computed EARLY from the
     one-hot column sums; only QM (SqS) and RT (stats) trail the DMA.
  3. loss:   per 512-col chunk, PSUM = RT^T @ fT + P128^T @ fT^2  (the second
     matmul realizes P[c]*sq_i since sum_d fT^2[d,i] = sq_i), then
     relu(PSUM + QM[c]) on the scalar engine and mask*accumulate on vector.
Total HBM traffic ~4.7MB/core (bf16 features in rows + transposed layouts,
prebuilt one-hots); every core computes redundantly (no collectives).
Note: DVE (MULTIPLY, BYPASS) tensor_scalar is pathologically slow (~3.5us
even for 16 elems) -- always pair mult with an add-0 second op instead.
"""

import numpy as np
import ml_dtypes

import concourse.bacc as bacc
import concourse.tile as tile
import concourse.mybir as mybir
from concourse.bass_utils import run_bass_kernel_spmd

N, D, K, NCORES = 8192, 128, 16, 8
T = N // 128               # 64 row-tiles of 128
NCH = 16                   # dot chunks of 512 cols
CH = N // NCH
FCH = 4                    # DMA / square chunking (2048 cols each)
EPS, MARGIN = 1e-6, 10.0
F32 = mybir.dt.float32
BF16 = mybir.dt.bfloat16
Alu = mybir.AluOpType
Act = mybir.ActivationFunctionType
AxX = mybir.AxisListType.X

_CACHE: dict = {}


def _build():
    if "nc" in _CACHE:
        return _CACHE["nc"]

    nc = bacc.Bacc("TRN2", target_bir_lowering=False, debug=False, num_devices=NCORES)
    fhr = nc.dram_tensor("fhr", [128, T * D], BF16, kind="ExternalInput").ap()
    ftr = nc.dram_tensor("ftr", [128, N], BF16, kind="ExternalInput").ap()
    eohr = nc.dram_tensor("eohr", [128, T * K], BF16, kind="ExternalInput").ap()
    eoht = nc.dram_tensor("eoht", [K, N], BF16, kind="ExternalInput").ap()
    res = nc.dram_tensor("res", [1, 1], F32, kind="ExternalOutput").ap()

    with tile.TileContext(nc) as tc:
        with (
            tc.tile_pool(name="sb", bufs=1) as sb,
            tc.tile_pool(name="ps", bufs=1, space="PSUM") as ps,
        ):
            # ---------------- loads (3 dispatch rings, eohs+fh first) -----------
            eohs = sb.tile([128, T * K], BF16)
            fh = sb.tile([128, T * D], BF16)
            ft = sb.tile([128, N], BF16)
            eohts = sb.tile([K, N], BF16)
            FC = T * D // FCH
            FT = N // FCH
            nc.sync.dma_start(eohs[:], eohr)
            nc.sync.dma_start(fh[:, 0 * FC:1 * FC], fhr[:, 0 * FC:1 * FC])
            nc.scalar.dma_start(fh[:, 1 * FC:2 * FC], fhr[:, 1 * FC:2 * FC])
            nc.sync.dma_start(fh[:, 2 * FC:3 * FC], fhr[:, 2 * FC:3 * FC])
            nc.scalar.dma_start(fh[:, 3 * FC:4 * FC], fhr[:, 3 * FC:4 * FC])
            nc.sync.dma_start(ft[:, 0 * FT:1 * FT], ftr[:, 0 * FT:1 * FT])
            nc.scalar.dma_start(ft[:, 1 * FT:2 * FT], ftr[:, 1 * FT:2 * FT])
            nc.gpsimd.dma_start(ft[:, 2 * FT:3 * FT], ftr[:, 2 * FT:3 * FT])
            nc.gpsimd.dma_start(ft[:, 3 * FT:4 * FT], ftr[:, 3 * FT:4 * FT])
            nc.scalar.dma_start(eohts[:], eoht)

            fh3 = fh.rearrange("p (t d) -> p t d", d=D)
            eoh3 = eohs.rearrange("p (t c) -> p t c", c=K)
            eoh3c = eohs.rearrange("p (t c) -> p c t", c=K)

            ones128 = sb.tile([128, 1], F32)
            nc.gpsimd.memset(ones128[:], 1.0)
            ones1 = sb.tile([1, 128], F32)
            nc.gpsimd.memset(ones1[:], 1.0)
            # preload the Relu activation table off the critical path
            dumm = sb.tile([1, 1], BF16)
            nc.scalar.activation(dumm[:], ones1[:, 0:1], Act.Relu)

            # ---------------- early: cnt and cnt-only coefficients --------------
            cntpart = sb.tile([128, K], F32)
            nc.vector.tensor_reduce(cntpart[:], eoh3c, axis=AxX, op=Alu.add)
            cntP = ps.tile([1, K], F32, tag="smallP", bufs=2, name="cntP")
            nc.tensor.matmul(cntP[:], ones128[:], cntpart[:], start=True, stop=True,
                             skip_group_check=True)
            cntf = sb.tile([1, K], F32)
            nc.vector.tensor_copy(cntf[:], cntP[:])

            alpha = sb.tile([1, K], F32)
            nc.vector.tensor_scalar(alpha[:], cntf[:], EPS - 1.0, None, op0=Alu.add)
            nc.vector.reciprocal(alpha[:], alpha[:])
            beta = sb.tile([1, K], F32)
            nc.vector.tensor_scalar(beta[:], cntf[:], -1.0, float(N) + EPS,
                                    op0=Alu.mult, op1=Alu.add)
            nc.vector.reciprocal(beta[:], beta[:])
            vmask = sb.tile([1, K], F32)
            nc.vector.tensor_scalar(vmask[:], cntf[:], 1.5, None, op0=Alu.is_ge)
            nmc = sb.tile([1, K], F32)
            nc.vector.tensor_scalar(nmc[:], cntf[:], -1.0, float(N),
                                    op0=Alu.mult, op1=Alu.add)        # N-cnt
            nc.vector.tensor_tensor(nmc[:], nmc[:], beta[:], op=Alu.mult)
            pf = sb.tile([1, K], F32)
            nc.vector.tensor_tensor(pf[:], cntf[:], alpha[:], op=Alu.mult)
            nc.vector.tensor_tensor(pf[:], pf[:], nmc[:], op=Alu.subtract)

            cpack = sb.tile([1, 3 * K], F32)
            nc.vector.tensor_scalar(cpack[:, 0:K], beta[:], 2.0, 0.0,
                                    op0=Alu.mult, op1=Alu.add)
            nc.vector.tensor_scalar(cpack[:, K:2 * K], alpha[:], -2.0, 0.0,
                                    op0=Alu.mult, op1=Alu.add)
            nc.vector.tensor_tensor(cpack[:, 2 * K:3 * K], pf[:], vmask[:],
                                    op=Alu.mult)                       # P*vm
            vm2 = cpack[:, 0:2 * K].rearrange("o (a c) -> o a c", c=K)
            vmb = vmask.unsqueeze(1).broadcast_to((1, 2, K))
            nc.vector.tensor_tensor(vm2[:, :, :], vm2, vmb, op=Alu.mult)

            # ---------------- squares ----------------
            # rows-layout squares (feed the SqS matmul chain)
            fsqs = []
            TPC = T // FCH
            for g in range(FCH):
                fsq = sb.tile([128, TPC * D], BF16, tag="fsq", bufs=4, name=f"fsq{g}")
                nc.scalar.activation(fsq[:], fh[:, g * FC:(g + 1) * FC], Act.Square)
                fsqs.append(fsq.rearrange("p (t d) -> p t d", d=D))
            # transposed squares (dot-phase MM2 rhs), split Pool/Act
            ft2 = sb.tile([128, N], BF16)
            for g in (0, 1):
                nc.gpsimd.tensor_tensor(ft2[:, g * FT:(g + 1) * FT],
                                        ft[:, g * FT:(g + 1) * FT],
                                        ft[:, g * FT:(g + 1) * FT], op=Alu.mult)
            for g in (2, 3):
                nc.scalar.activation(ft2[:, g * FT:(g + 1) * FT],
                                     ft[:, g * FT:(g + 1) * FT], Act.Square)

            # ---------------- stats + sqstats chains (+ bcast matmul) -----------
            statsP = ps.tile([128, K], F32)
            sqstP = ps.tile([128, K], F32)
            bcP = ps.tile([128, 3 * K], F32)
            for t in range(T):
                nc.tensor.matmul(statsP[:], fh3[:, t, :], eoh3[:, t, :],
                                 start=(t == 0), stop=(t == T - 1),
                                 skip_group_check=True)
                nc.tensor.matmul(sqstP[:], fsqs[t // TPC][:, t % TPC, :],
                                 eoh3[:, t, :],
                                 start=(t == 0), stop=(t == T - 1),
                                 skip_group_check=True)
                if t == 31:
                    nc.tensor.matmul(bcP[:], ones1[:], cpack[:], start=True,
                                     stop=True, skip_group_check=True)

            # ---------------- SqS = column sums of sqstats ----------------
            sqstS = sb.tile([128, K], F32)
            nc.vector.tensor_copy(sqstS[:], sqstP[:])
            csP = ps.tile([1, K], F32, tag="smallP", bufs=2, name="csP")
            nc.tensor.matmul(csP[:], ones128[:], sqstS[:], start=True, stop=True,
                             skip_group_check=True)
            SqS = sb.tile([1, K], F32)
            nc.vector.tensor_copy(SqS[:], csP[:])

            # ---------------- QM (needs SqS) ----------------
            ssall = sb.tile([1, 1], F32)
            nc.vector.tensor_reduce(ssall[:], SqS[:], axis=AxX, op=Alu.add)
            t1 = sb.tile([1, K], F32)
            nc.vector.scalar_tensor_tensor(t1[:], SqS[:], -1.0,
                                           ssall.broadcast_to((1, K)),
                                           op0=Alu.mult, op1=Alu.add)  # SSall-SqS
            nc.vector.tensor_tensor(t1[:], t1[:], beta[:], op=Alu.mult)
            qm = sb.tile([1, K], F32)
            nc.vector.tensor_tensor(qm[:], SqS[:], alpha[:], op=Alu.mult)
            nc.vector.scalar_tensor_tensor(qm[:], qm[:], MARGIN, t1[:],
                                           op0=Alu.add, op1=Alu.subtract)
            nc.vector.tensor_tensor(qm[:], qm[:], vmask[:], op=Alu.mult)
            qmtP = ps.tile([K, 1], F32, tag="smallP", bufs=2, name="qmtP")
            nc.tensor.matmul(qmtP[:], qm[:], ones1[:, 0:1], start=True, stop=True,
                             skip_group_check=True)
            qm16 = sb.tile([K, 1], F32)
            nc.vector.tensor_copy(qm16[:], qmtP[:])

            # ---------------- RT (needs stats) ----------------
            statsS = sb.tile([128, K], F32)
            nc.vector.tensor_copy(statsS[:], statsP[:])
            ftot = sb.tile([128, 1], F32)
            nc.vector.tensor_reduce(ftot[:], statsS[:], axis=AxX, op=Alu.add)
            rtf = sb.tile([128, K], F32)
            nc.vector.tensor_tensor(rtf[:], ftot.broadcast_to((128, K)), statsS[:],
                                    op=Alu.subtract)                   # Ftot-C^T
            nc.vector.tensor_tensor(rtf[:], rtf[:], bcP[:, 0:K], op=Alu.mult)
            tmp2 = sb.tile([128, K], F32)
            nc.vector.tensor_tensor(tmp2[:], statsS[:], bcP[:, K:2 * K], op=Alu.mult)
            rts = sb.tile([128, K], BF16)
            nc.vector.tensor_tensor(rts[:], rtf[:], tmp2[:], op=Alu.add)
            p128s = sb.tile([128, K], BF16)
            nc.vector.tensor_copy(p128s[:], bcP[:, 2 * K:3 * K])

            # ---------------- loss chunks ----------------
            partials = sb.tile([K, NCH], F32)
            for k in range(NCH):
                dP = ps.tile([K, CH], F32, tag="dpsum", bufs=3, name=f"dP{k}")
                nc.tensor.matmul(dP[:], rts[:], ft[:, k * CH:(k + 1) * CH],
                                 start=True, stop=False)
                nc.tensor.matmul(dP[:], p128s[:], ft2[:, k * CH:(k + 1) * CH],
                                 start=False, stop=True)
                mskd = sb.tile([K, CH], BF16, tag="mskd", bufs=3, name=f"m{k}")
                nc.scalar.activation(mskd[:], dP[:], Act.Relu, bias=qm16[:])
                scr = sb.tile([K, CH], BF16, tag="scr", bufs=3, name=f"s{k}")
                nc.vector.scalar_tensor_tensor(scr[:], mskd[:], 0.0,
                                               eohts[:, k * CH:(k + 1) * CH],
                                               op0=Alu.add, op1=Alu.mult,
                                               accum_out=partials[:, k:k + 1])

            # ---------------- final reduction ----------------
            numP = ps.tile([1, NCH], F32, tag="smallP", bufs=2, name="numP")
            nc.tensor.matmul(numP[:], ones128[0:K, :], partials[:],
                             start=True, stop=True, skip_group_check=True)
            num = sb.tile([1, 1], F32)
            nc.vector.tensor_reduce(num[:], numP[:], axis=AxX, op=Alu.add)
            dv = sb.tile([1, K], F32)
            nc.vector.tensor_tensor(dv[:], cntf[:], vmask[:], op=Alu.mult)
            den = sb.tile([1, 1], F32)
            nc.vector.tensor_reduce(den[:], dv[:], axis=AxX, op=Alu.add)
            nc.vector.tensor_scalar(den[:], den[:], 1.0, None, op0=Alu.max)
            nc.vector.reciprocal(den[:], den[:])
            resS = sb.tile([1, 1], F32)
            nc.vector.tensor_tensor(resS[:], num[:], den[:], op=Alu.mult)
            nc.sync.dma_start(res, resS[:])

    nc.compile()
    _CACHE["nc"] = nc
    return nc


def _make_in_maps(features, labels):
    feats = np.ascontiguousarray(np.asarray(features, dtype=np.float32))
    lab = np.ascontiguousarray(np.asarray(labels)).astype(np.int64)
    bf = ml_dtypes.bfloat16

    oh = (lab[:, None] == np.arange(K, dtype=np.int64)[None, :]).astype(bf)  # (N, K)
    one = {
        "fhr": np.ascontiguousarray(
            feats.reshape(T, 128, D).transpose(1, 0, 2).reshape(128, T * D)
        ).astype(bf),
        "ftr": np.ascontiguousarray(feats.T).astype(bf),
        "eohr": np.ascontiguousarray(
            oh.reshape(T, 128, K).transpose(1, 0, 2).reshape(128, T * K)),
        "eoht": np.ascontiguousarray(oh.T),
    }
    return [dict(one) for _ in range(NCORES)]


def kernel(features, labels):
    nc = _build()
    in_maps = _make_in_maps(features, labels)
    out = run_bass_kernel_spmd(nc, in_maps, core_ids=list(range(NCORES)))
    return np.float32(out.results[0]["res"][0, 0])
